# revision 18
# baseline (speedup 1.0000x reference)
"""Trainium2 Bass kernel for nn_Agent_88828513616344 (gnn_message_passing).

Math (see reference): with A the [208,208] block matrix built from tp_w
(CG-scaled, block-diagonal per irrep l, diagonal in m), the pairwise tensor
product scores factorize as

    features = (x @ A) @ fragment_environments.T          # [N, F]
    actions  = argmax(where(masks, features, MIN) + g)    # gumbel-max, fixed key
    value    = features @ critic_w + critic_b = (x @ A) @ (fragment_environments.T @ critic_w) + b

Sharding: N=4096 rows split across 8 NeuronCores (data parallel); the
fragment-environment table, tp weights and critic are replicated.

Per-core device kernel (all sizes per core: 512 rows):
  - PE computes t = g + x' @ y^T into PSUM in bf16 (g is injected through an
    identity matmul so no vector-engine add is needed; PSUM accumulates fp32).
  - ScalarE copies PSUM tiles into a full [128, 8192] SBUF row buffer.
  - VectorE folds the row twice with fused pairwise max (tensor_tensor_reduce):
    8192 -> 4096 -> 2048 "quad maxes", then top-8 + indices (Max/MaxIndex).
  - The critic head (tiny [208]-contraction) runs in true fp32 on PE.
Device returns per row the top-8 quad indices (each quad = 4 candidate
columns). The host rescores those <=32 candidate columns per row exactly in
float64 — the bf16 feature error (~3e-3) is astronomically unlikely to push
the true gumbel-max winner out of the device's top-8 quads (the gumbel top-9
spread is O(1)), and the final argmax over candidates is exact.
"""

import numpy as np

MUL = 13
LMAX = 3
FAN_IN = 4 * MUL * MUL
N, F, D = 4096, 8192, 208
NCORES = 8
RPC = N // NCORES  # rows per core = 512
ICH = RPC // 128  # i-chunks per core = 4
JT = 512
NJT = F // JT  # 16
KSPLIT = 104  # 208 = 104 + 104 contraction split
F32MIN = float(np.finfo(np.float32).min)
MASKPEN = -3.0e38  # large-negative, finite in bf16 (avoids 0*inf NaN in the PE)

_cache: dict = {}


def _gumbel() -> np.ndarray:
    """The reference's deterministic gumbel noise (fixed key, fixed shape).

    Computed with the same call on the same (default) jax backend as the
    reference, so the bits match exactly — PRNG output differs between
    backends (trn2 vs cpu), so do NOT pin a device here.
    """
    if "g" not in _cache:
        import jax

        g = jax.random.gumbel(jax.random.key(42), (N, F), dtype=np.float32)
        _cache["g"] = np.asarray(g)
    return _cache["g"]


def _build_A(tp_w: np.ndarray) -> np.ndarray:
    """[208, 208] float64 block matrix: A[off+u*m+t, off+v*m+t] = c_l W_l[u,v]."""
    A = np.zeros((D, D), np.float64)
    off = 0
    for l in range(LMAX + 1):
        m = 2 * l + 1
        c = 1.0 / np.sqrt((2 * l + 1) * FAN_IN)
        W = tp_w[l].astype(np.float64) * c
        for t in range(m):
            A[off + t : off + MUL * m : m, off + t : off + MUL * m : m] = W
        off += MUL * m
    return A


def _pin_dma_lanes():
    """Pin HWDGE DMA-completion sems to one lane per issuing ring.

    Tile round-robins DMA completions over 8 DMAHW semaphore lanes, so an
    instruction first-consuming two DMA'd tiles needs two sync waits — but
    walrus rejects Matmult instructions with more than one sync wait ("Too
    many sync wait commands" in setupSyncWait<S3_LW>). HWDGE executes DMAs
    FIFO per issuing engine (two physical rings: SP and ACT), so pinning each
    ring to a single monotone lane is safe and collapses all DMA waits from
    one ring into a single `lane >= tick` wait.
    """
    if _cache.get("lanes_patched"):
        return
    import concourse.mybir as mybir
    import concourse.tile_sem_assignment as tsa
    from concourse.tile_scheduler import DMAInst

    lanes = {mybir.EngineType.SP: 0, mybir.EngineType.Activation: 1}
    orig = tsa.TileClockTick._assign_tick

    def pinned(self, inst):
        if isinstance(inst, DMAInst):
            lane = lanes.get(inst.engine)
            if lane is not None:
                self.next_hw_dma_idx = lane
        return orig(self, inst)

    tsa.TileClockTick._assign_tick = pinned
    _cache["lanes_patched"] = True


def _split_multi_waits(nc):
    """Walrus codegen allows only ONE sync wait per ISA instruction in this
    build. Move surplus waits of any scheduled instruction onto engine-local
    NoOps inserted directly before it (same engine stream => same semantics).
    """
    import concourse.mybir as mybir

    fn = nc.m.functions[0]
    k = 0
    for blk in fn.blocks:
        out = []
        for inst in blk.instructions:
            si = inst.sync_info
            waits = list(si.on_wait) if si is not None and si.on_wait else []
            if len(waits) > 1:
                for w in waits[:-1]:
                    nop = mybir.InstNoOp(name=f"I-wsplit-{k}", ins=[], outs=[])
                    k += 1
                    nop.engine = inst.engine
                    nop.sync_info = mybir.SyncInfo(on_wait=[w], on_update=[])
                    out.append(nop)
                inst.sync_info = mybir.SyncInfo(
                    on_wait=[waits[-1]], on_update=list(si.on_update or [])
                )
            out.append(inst)
        blk.instructions = out


def _build_nc(reps: int = 1, split_waits: bool = True):
    import concourse.bass as bass
    import concourse.mybir as mybir
    import concourse.tile as tile

    _pin_dma_lanes()

    f32 = mybir.dt.float32
    bf16 = mybir.dt.bfloat16
    u32 = mybir.dt.uint32
    MAX = mybir.AluOpType.max

    nc = bass.Bass("TRN2", target_bir_lowering=False, debug=False)
    xb = nc.dram_tensor("xb", [D, RPC], bf16, kind="ExternalInput").ap()
    yb = nc.dram_tensor("yb", [D, F], bf16, kind="ExternalInput").ap()
    gb = nc.dram_tensor("gb", [RPC, F], bf16, kind="ExternalInput").ap()
    xf = nc.dram_tensor("xf", [D, RPC], f32, kind="ExternalInput").ap()
    wp = nc.dram_tensor("wp", [D, 1], f32, kind="ExternalInput").ap()
    identb = nc.dram_tensor("identb", [128, 128], bf16, kind="ExternalInput").ap()
    cand = nc.dram_tensor("cand", [RPC, 8], u32, kind="ExternalOutput").ap()
    val = nc.dram_tensor("val", [RPC, 1], f32, kind="ExternalOutput").ap()

    YC = 4  # load y in column chunks so early matmuls start sooner

    with tile.TileContext(nc) as tc:
        with (
            tc.tile_pool(name="const", bufs=1) as cpool,
            tc.tile_pool(name="gpool", bufs=3) as gpool,
            tc.tile_pool(name="tpool", bufs=2) as tpool,
            tc.tile_pool(name="hpool", bufs=2) as hpool,
            tc.tile_pool(name="small", bufs=3) as spool,
            tc.tile_pool(name="psum", bufs=5, space="PSUM") as ppool,
            tc.tile_pool(name="psumv", bufs=2, space="PSUM") as pvpool,
            tc.tile_pool(name="psumscr", bufs=1, space="PSUM") as pspool,
        ):
            y0 = cpool.tile([KSPLIT, F], bf16, tag="y0")
            y1 = cpool.tile([KSPLIT, F], bf16, tag="y1")
            x0 = cpool.tile([KSPLIT, RPC], bf16, tag="x0")
            x1 = cpool.tile([KSPLIT, RPC], bf16, tag="x1")
            xf0 = cpool.tile([KSPLIT, RPC], f32, tag="xf0")
            xf1 = cpool.tile([KSPLIT, RPC], f32, tag="xf1")
            w0 = cpool.tile([KSPLIT, 1], f32, tag="w0")
            w1 = cpool.tile([KSPLIT, 1], f32, tag="w1")
            Ib = cpool.tile([128, 128], bf16, tag="Ib")

            FC = F // YC
            for _rep in range(reps):
                # constants + y on the ACT HWDGE ring; g stream owns the SP ring
                nc.scalar.dma_start(Ib[:], identb[:, :])
                nc.scalar.dma_start(x0[:], xb[0:KSPLIT, :])
                nc.scalar.dma_start(x1[:], xb[KSPLIT : 2 * KSPLIT, :])
                cs0 = slice(0, FC)
                nc.scalar.dma_start(y0[:, cs0], yb[0:KSPLIT, cs0])
                nc.scalar.dma_start(y1[:, cs0], yb[KSPLIT : 2 * KSPLIT, cs0])
                nc.scalar.dma_start(xf0[:], xf[0:KSPLIT, :])
                nc.scalar.dma_start(xf1[:], xf[KSPLIT : 2 * KSPLIT, :])
                nc.scalar.dma_start(w0[:], wp[0:KSPLIT, :])
                nc.scalar.dma_start(w1[:], wp[KSPLIT : 2 * KSPLIT, :])
                for jc in range(1, YC):
                    cs = slice(jc * FC, (jc + 1) * FC)
                    nc.scalar.dma_start(y0[:, cs], yb[0:KSPLIT, cs])
                    nc.scalar.dma_start(y1[:, cs], yb[KSPLIT : 2 * KSPLIT, cs])

                scr = pspool.tile([128, 1], f32, tag="scr")
                for ic in range(ICH):
                    rs = slice(ic * 128, (ic + 1) * 128)
                    gt = gpool.tile([128, F], bf16, tag="gt")
                    nc.sync.dma_start(gt[:], gb[rs, :])

                    # observer matmul: absorbs the g-stream's DMA-lane wait
                    # into a dedicated never-read PSUM bank, so no real
                    # matmul ever needs two sync waits (walrus limit).
                    nc.tensor.matmul(
                        scr[:], gt[:, 0:128], gt[:, 0:1], start=True, stop=True
                    )

                    # critic head in true fp32 (exact): val = x'[rows] @ wp
                    pv = pvpool.tile([128, 1], f32, tag="pv")
                    nc.tensor.matmul(pv[:], xf0[:, rs], w0[:], start=True, stop=False)
                    nc.tensor.matmul(pv[:], xf1[:, rs], w1[:], start=False, stop=True)
                    vt = spool.tile([128, 1], f32, tag="vt")
                    nc.scalar.copy(vt[:], pv[:])
                    nc.scalar.dma_start(val[rs, :], vt[:])

                    # t = g + x' @ y^T (bf16 inputs, fp32 PSUM accumulation).
                    # g-inject matmul goes FIRST (start=True) so the PSUM
                    # WAR wait lands on an instruction with no fresh-DMA wait.
                    t = tpool.tile([128, F], bf16, tag="t")
                    for j in range(NJT):
                        js = slice(j * JT, (j + 1) * JT)
                        pt = ppool.tile([128, JT], f32, tag="pt")
                        nc.tensor.matmul(
                            pt[:], Ib[:], gt[:, js], start=True, stop=False
                        )
                        nc.tensor.matmul(
                            pt[:], x0[:, rs], y0[:, js], start=False, stop=False
                        )
                        nc.tensor.matmul(
                            pt[:], x1[:, rs], y1[:, js], start=False, stop=True
                        )
                        nc.scalar.copy(t[:, js], pt[:])

                    # adjacent-pair max fold (bf16 tensor_tensor -> 2x DVE mode)
                    #   h[a*512+o]  = max(t[2a*512+o],  t[(2a+1)*512+o])
                    #   h2[b*512+o] = max(h[2b*512+o],  h[(2b+1)*512+o])
                    #   h3[c*512+o] = max(h2[2c*512+o], h2[(2c+1)*512+o])
                    # -> oct r covers columns (8*(r//512)+k)*512 + r%512, k<8
                    h = hpool.tile([128, F // 2], bf16, tag="h")
                    for a in range(8):
                        nc.vector.tensor_tensor(
                            h[:, a * JT : (a + 1) * JT],
                            t[:, (2 * a) * JT : (2 * a + 1) * JT],
                            t[:, (2 * a + 1) * JT : (2 * a + 2) * JT],
                            MAX,
                        )
                    h2 = hpool.tile([128, F // 4], bf16, tag="h2")
                    for b in range(4):
                        nc.vector.tensor_tensor(
                            h2[:, b * JT : (b + 1) * JT],
                            h[:, (2 * b) * JT : (2 * b + 1) * JT],
                            h[:, (2 * b + 1) * JT : (2 * b + 2) * JT],
                            MAX,
                        )
                    h3 = hpool.tile([128, F // 8], bf16, tag="h3")
                    for c in range(2):
                        nc.vector.tensor_tensor(
                            h3[:, c * JT : (c + 1) * JT],
                            h2[:, (2 * c) * JT : (2 * c + 1) * JT],
                            h2[:, (2 * c + 1) * JT : (2 * c + 2) * JT],
                            MAX,
                        )
                    m8 = spool.tile([128, 8], bf16, tag="m8")
                    nc.vector.max(out=m8[:], in_=h3[:])
                    i8 = spool.tile([128, 8], u32, tag="i8")
                    nc.vector.max_index(out=i8[:], in_max=m8[:], in_values=h3[:])
                    nc.scalar.dma_start(cand[rs, :], i8[:])
    if split_waits:
        _split_multi_waits(nc)
    return nc


def _get_nc():
    if "nc" not in _cache:
        _cache["nc"] = _build_nc()
    return _cache["nc"]


LAST_RESULTS = None  # BassKernelResults of the most recent device run


def kernel(x, fragment_environments, tp_w, critic_w, critic_b, masks):
    global LAST_RESULTS
    import ml_dtypes

    from concourse.bass_utils import run_bass_kernel_spmd

    bf16 = ml_dtypes.bfloat16

    x = np.asarray(x, np.float32)
    y = np.asarray(fragment_environments, np.float32)
    tp_w = np.asarray(tp_w, np.float32)
    critic_w = np.asarray(critic_w, np.float32)
    critic_b = np.asarray(critic_b, np.float32)
    masks = np.asarray(masks)

    # ---- host-side prep (tiny vs the [N, F] device work) ----
    A = _build_A(tp_w)  # [208, 208] f64
    xp64 = x.astype(np.float64) @ A  # [N, 208] f64
    xpT32 = np.ascontiguousarray(xp64.T.astype(np.float32))  # [208, N]
    xpTb = xpT32.astype(bf16)
    yTb = np.ascontiguousarray(y.T).astype(bf16)  # [208, F]
    wp = np.ascontiguousarray(
        (y.astype(np.float64).T @ critic_w.astype(np.float64)).astype(np.float32)
    )  # [208, 1]
    g = _gumbel()
    all_valid = bool(masks.all())
    if all_valid:
        gmb = g.astype(bf16)
    else:
        gmb = np.where(masks, g, np.float32(MASKPEN)).astype(bf16)
    identity = np.eye(128, dtype=np.float32).astype(bf16)

    in_maps = []
    for c in range(NCORES):
        rows = slice(c * RPC, (c + 1) * RPC)
        in_maps.append(
            {
                "xb": np.ascontiguousarray(xpTb[:, rows]),
                "yb": yTb,
                "gb": np.ascontiguousarray(gmb[rows]),
                "xf": np.ascontiguousarray(xpT32[:, rows]),
                "wp": wp,
                "identb": identity,
            }
        )

    nc = _get_nc()
    res = run_bass_kernel_spmd(nc, in_maps, core_ids=list(range(NCORES)))
    LAST_RESULTS = res

    cand = np.concatenate([r["cand"] for r in res.results], axis=0)  # [N, 8] u32
    value = np.concatenate([r["val"] for r in res.results], axis=0)  # [N, 1] f32

    # ---- host rescore of the <=64 candidate columns per row (exact f64) ----
    q = cand.astype(np.int64)
    q[q >= F // 8] = 0  # guard (unmatched top-8 slots would be 0xFFFFFFFF)
    # oct q covers columns (8*(q//512) + k)*512 + q%512, k in 0..7
    js = (
        (8 * (q[:, :, None] // JT) + np.arange(8)[None, None, :]) * JT
        + q[:, :, None] % JT
    ).reshape(N, 64)
    rows = np.arange(N)[:, None]
    f_cand = np.einsum(
        "nd,ncd->nc", xp64, y.astype(np.float64)[js], optimize=True
    )  # [N, 32] f64
    g_cand = g[rows, js].astype(np.float64)
    if all_valid:
        score = f_cand + g_cand
    else:
        m_cand = masks[rows, js]
        score = np.where(m_cand, f_cand, np.float64(F32MIN)) + g_cand
    best = score.max(axis=1, keepdims=True)
    jpick = np.where(score == best, js, np.int64(1 << 40))
    actions = jpick.min(axis=1).astype(np.int32)

    value = value + critic_b.astype(np.float32)  # [N, 1]
    return actions, value


if __name__ == "__main__":
    # smoke test with random data of the right shapes
    rng = np.random.default_rng(0)
    ins = {
        "x": rng.standard_normal((N, D)).astype(np.float32),
        "fragment_environments": rng.standard_normal((F, D)).astype(np.float32),
        "tp_w": rng.standard_normal((4, MUL, MUL)).astype(np.float32),
        "critic_w": (rng.standard_normal((F, 1)) / np.sqrt(F)).astype(np.float32),
        "critic_b": np.zeros((1,), dtype=np.float32),
        "masks": np.ones((N, F), dtype=bool),
    }
    a, v = kernel(**ins)
    print(a[:8], v[:4, 0])


# revision 19
# speedup vs baseline: 35.8394x; 35.8394x over previous
"""Trainium2 Bass kernel for nn_Agent_88828513616344 (gnn_message_passing).

Math (see reference): with A the [208,208] block matrix built from tp_w
(CG-scaled, block-diagonal per irrep l, diagonal in m), the pairwise tensor
product scores factorize as

    features = (x @ A) @ fragment_environments.T          # [N, F]
    actions  = argmax(where(masks, features, MIN) + g)    # gumbel-max, fixed key
    value    = features @ critic_w + critic_b = (x @ A) @ (fragment_environments.T @ critic_w) + b

Sharding: N=4096 rows split across 8 NeuronCores (data parallel); the
fragment-environment table, tp weights and critic are replicated.

Per-core device kernel (all sizes per core: 512 rows):
  - PE computes t = g + x' @ y^T into PSUM in bf16 (g is injected through an
    identity matmul so no vector-engine add is needed; PSUM accumulates fp32).
  - ScalarE copies PSUM tiles into a full [128, 8192] SBUF row buffer.
  - VectorE folds the row twice with fused pairwise max (tensor_tensor_reduce):
    8192 -> 4096 -> 2048 "quad maxes", then top-8 + indices (Max/MaxIndex).
  - The critic head (tiny [208]-contraction) runs in true fp32 on PE.
Device returns per row the top-8 quad indices (each quad = 4 candidate
columns). The host rescores those <=32 candidate columns per row exactly in
float64 — the bf16 feature error (~3e-3) is astronomically unlikely to push
the true gumbel-max winner out of the device's top-8 quads (the gumbel top-9
spread is O(1)), and the final argmax over candidates is exact.
"""

import numpy as np

MUL = 13
LMAX = 3
FAN_IN = 4 * MUL * MUL
N, F, D = 4096, 8192, 208
NCORES = 8
RPC = N // NCORES  # rows per core = 512
ICH = RPC // 128  # i-chunks per core = 4
JT = 512
NJT = F // JT  # 16
KSPLIT = 104  # 208 = 104 + 104 contraction split
F32MIN = float(np.finfo(np.float32).min)
MASKPEN = -3.0e38  # large-negative, finite in bf16 (avoids 0*inf NaN in the PE)

_cache: dict = {}


def _gumbel() -> np.ndarray:
    """The reference's deterministic gumbel noise (fixed key, fixed shape).

    Computed with the same call on the same (default) jax backend as the
    reference, so the bits match exactly — PRNG output differs between
    backends (trn2 vs cpu), so do NOT pin a device here.
    """
    if "g" not in _cache:
        import jax

        g = jax.random.gumbel(jax.random.key(42), (N, F), dtype=np.float32)
        _cache["g"] = np.asarray(g)
    return _cache["g"]


def _build_A(tp_w: np.ndarray) -> np.ndarray:
    """[208, 208] float64 block matrix: A[off+u*m+t, off+v*m+t] = c_l W_l[u,v]."""
    A = np.zeros((D, D), np.float64)
    off = 0
    for l in range(LMAX + 1):
        m = 2 * l + 1
        c = 1.0 / np.sqrt((2 * l + 1) * FAN_IN)
        W = tp_w[l].astype(np.float64) * c
        for t in range(m):
            A[off + t : off + MUL * m : m, off + t : off + MUL * m : m] = W
        off += MUL * m
    return A


def _pin_dma_lanes():
    """Pin HWDGE DMA-completion sems to one lane per issuing ring.

    Tile round-robins DMA completions over 8 DMAHW semaphore lanes, so an
    instruction first-consuming two DMA'd tiles needs two sync waits — but
    walrus rejects Matmult instructions with more than one sync wait ("Too
    many sync wait commands" in setupSyncWait<S3_LW>). HWDGE executes DMAs
    FIFO per issuing engine (two physical rings: SP and ACT), so pinning each
    ring to a single monotone lane is safe and collapses all DMA waits from
    one ring into a single `lane >= tick` wait.
    """
    if _cache.get("lanes_patched"):
        return
    import concourse.mybir as mybir
    import concourse.tile_sem_assignment as tsa
    from concourse.tile_scheduler import DMAInst

    lanes = {mybir.EngineType.SP: 0, mybir.EngineType.Activation: 1}
    orig = tsa.TileClockTick._assign_tick

    def pinned(self, inst):
        if isinstance(inst, DMAInst):
            lane = lanes.get(inst.engine)
            if lane is not None:
                self.next_hw_dma_idx = lane
        return orig(self, inst)

    tsa.TileClockTick._assign_tick = pinned
    _cache["lanes_patched"] = True


def _split_multi_waits(nc):
    """Walrus codegen allows only ONE sync wait per ISA instruction in this
    build. Move surplus waits of any scheduled instruction onto engine-local
    NoOps inserted directly before it (same engine stream => same semantics).
    """
    import concourse.mybir as mybir

    fn = nc.m.functions[0]
    k = 0
    for blk in fn.blocks:
        out = []
        for inst in blk.instructions:
            si = inst.sync_info
            waits = list(si.on_wait) if si is not None and si.on_wait else []
            if len(waits) > 1:
                for w in waits[:-1]:
                    nop = mybir.InstNoOp(name=f"I-wsplit-{k}", ins=[], outs=[])
                    k += 1
                    nop.engine = inst.engine
                    nop.sync_info = mybir.SyncInfo(on_wait=[w], on_update=[])
                    out.append(nop)
                inst.sync_info = mybir.SyncInfo(
                    on_wait=[waits[-1]], on_update=list(si.on_update or [])
                )
            out.append(inst)
        blk.instructions = out


def _build_nc(reps: int = 1, split_waits: bool = True):
    import concourse.bass as bass
    import concourse.mybir as mybir
    import concourse.tile as tile

    _pin_dma_lanes()

    f32 = mybir.dt.float32
    bf16 = mybir.dt.bfloat16
    u32 = mybir.dt.uint32
    MAX = mybir.AluOpType.max

    nc = bass.Bass("TRN2", target_bir_lowering=False, debug=False)
    xb = nc.dram_tensor("xb", [D, RPC], bf16, kind="ExternalInput").ap()
    yb = nc.dram_tensor("yb", [D, F], bf16, kind="ExternalInput").ap()
    gb = nc.dram_tensor("gb", [RPC, F], bf16, kind="ExternalInput").ap()
    xf = nc.dram_tensor("xf", [D, RPC], f32, kind="ExternalInput").ap()
    wp = nc.dram_tensor("wp", [D, 1], f32, kind="ExternalInput").ap()
    identb = nc.dram_tensor("identb", [128, 128], bf16, kind="ExternalInput").ap()
    cand = nc.dram_tensor("cand", [RPC, 8], u32, kind="ExternalOutput").ap()
    val = nc.dram_tensor("val", [RPC, 1], f32, kind="ExternalOutput").ap()

    YC = 4  # load y in column chunks so early matmuls start sooner

    with tile.TileContext(nc) as tc:
        with (
            tc.tile_pool(name="const", bufs=1) as cpool,
            tc.tile_pool(name="gpool", bufs=3) as gpool,
            tc.tile_pool(name="tpool", bufs=2) as tpool,
            tc.tile_pool(name="hpool", bufs=2) as hpool,
            tc.tile_pool(name="small", bufs=3) as spool,
            tc.tile_pool(name="psum", bufs=5, space="PSUM") as ppool,
            tc.tile_pool(name="psumv", bufs=2, space="PSUM") as pvpool,
            tc.tile_pool(name="psumscr", bufs=1, space="PSUM") as pspool,
        ):
            y0 = cpool.tile([KSPLIT, F], bf16, tag="y0")
            y1 = cpool.tile([KSPLIT, F], bf16, tag="y1")
            x0 = cpool.tile([KSPLIT, RPC], bf16, tag="x0")
            x1 = cpool.tile([KSPLIT, RPC], bf16, tag="x1")
            xf0 = cpool.tile([KSPLIT, RPC], f32, tag="xf0")
            xf1 = cpool.tile([KSPLIT, RPC], f32, tag="xf1")
            w0 = cpool.tile([KSPLIT, 1], f32, tag="w0")
            w1 = cpool.tile([KSPLIT, 1], f32, tag="w1")
            Ib = cpool.tile([128, 128], bf16, tag="Ib")

            FC = F // YC
            for _rep in range(reps):
                # constants + y on the ACT HWDGE ring; g stream owns the SP ring
                nc.scalar.dma_start(Ib[:], identb[:, :])
                nc.scalar.dma_start(x0[:], xb[0:KSPLIT, :])
                nc.scalar.dma_start(x1[:], xb[KSPLIT : 2 * KSPLIT, :])
                cs0 = slice(0, FC)
                nc.scalar.dma_start(y0[:, cs0], yb[0:KSPLIT, cs0])
                nc.scalar.dma_start(y1[:, cs0], yb[KSPLIT : 2 * KSPLIT, cs0])
                nc.scalar.dma_start(xf0[:], xf[0:KSPLIT, :])
                nc.scalar.dma_start(xf1[:], xf[KSPLIT : 2 * KSPLIT, :])
                nc.scalar.dma_start(w0[:], wp[0:KSPLIT, :])
                nc.scalar.dma_start(w1[:], wp[KSPLIT : 2 * KSPLIT, :])
                for jc in range(1, YC):
                    cs = slice(jc * FC, (jc + 1) * FC)
                    nc.scalar.dma_start(y0[:, cs], yb[0:KSPLIT, cs])
                    nc.scalar.dma_start(y1[:, cs], yb[KSPLIT : 2 * KSPLIT, cs])

                scr = pspool.tile([128, 1], f32, tag="scr")
                for ic in range(ICH):
                    rs = slice(ic * 128, (ic + 1) * 128)
                    gt = gpool.tile([128, F], bf16, tag="gt")
                    nc.sync.dma_start(gt[:], gb[rs, :])

                    # observer matmul: absorbs the g-stream's DMA-lane wait
                    # into a dedicated never-read PSUM bank, so no real
                    # matmul ever needs two sync waits (walrus limit).
                    nc.tensor.matmul(
                        scr[:], gt[:, 0:128], gt[:, 0:1], start=True, stop=True
                    )

                    # critic head in true fp32 (exact): val = x'[rows] @ wp
                    pv = pvpool.tile([128, 1], f32, tag="pv")
                    nc.tensor.matmul(pv[:], xf0[:, rs], w0[:], start=True, stop=False)
                    nc.tensor.matmul(pv[:], xf1[:, rs], w1[:], start=False, stop=True)
                    vt = spool.tile([128, 1], f32, tag="vt")
                    nc.scalar.copy(vt[:], pv[:])
                    nc.scalar.dma_start(val[rs, :], vt[:])

                    # t = g + x' @ y^T (bf16 inputs, fp32 PSUM accumulation).
                    # g-inject matmul goes FIRST (start=True) so the PSUM
                    # WAR wait lands on an instruction with no fresh-DMA wait.
                    t = tpool.tile([128, F], bf16, tag="t")
                    for j in range(NJT):
                        js = slice(j * JT, (j + 1) * JT)
                        pt = ppool.tile([128, JT], f32, tag="pt")
                        nc.tensor.matmul(
                            pt[:], Ib[:], gt[:, js], start=True, stop=False
                        )
                        nc.tensor.matmul(
                            pt[:], x0[:, rs], y0[:, js], start=False, stop=False
                        )
                        nc.tensor.matmul(
                            pt[:], x1[:, rs], y1[:, js], start=False, stop=True
                        )
                        nc.scalar.copy(t[:, js], pt[:])

                    # adjacent-pair max fold (bf16 tensor_tensor -> 2x DVE mode)
                    #   h[a*512+o]  = max(t[2a*512+o],  t[(2a+1)*512+o])
                    #   h2[b*512+o] = max(h[2b*512+o],  h[(2b+1)*512+o])
                    #   h3[c*512+o] = max(h2[2c*512+o], h2[(2c+1)*512+o])
                    # -> oct r covers columns (8*(r//512)+k)*512 + r%512, k<8
                    h = hpool.tile([128, F // 2], bf16, tag="h")
                    for a in range(8):
                        nc.vector.tensor_tensor(
                            h[:, a * JT : (a + 1) * JT],
                            t[:, (2 * a) * JT : (2 * a + 1) * JT],
                            t[:, (2 * a + 1) * JT : (2 * a + 2) * JT],
                            MAX,
                        )
                    h2 = hpool.tile([128, F // 4], bf16, tag="h2")
                    for b in range(4):
                        nc.vector.tensor_tensor(
                            h2[:, b * JT : (b + 1) * JT],
                            h[:, (2 * b) * JT : (2 * b + 1) * JT],
                            h[:, (2 * b + 1) * JT : (2 * b + 2) * JT],
                            MAX,
                        )
                    h3 = hpool.tile([128, F // 8], bf16, tag="h3")
                    for c in range(2):
                        nc.vector.tensor_tensor(
                            h3[:, c * JT : (c + 1) * JT],
                            h2[:, (2 * c) * JT : (2 * c + 1) * JT],
                            h2[:, (2 * c + 1) * JT : (2 * c + 2) * JT],
                            MAX,
                        )
                    m8 = spool.tile([128, 8], bf16, tag="m8")
                    nc.vector.max(out=m8[:], in_=h3[:])
                    i8 = spool.tile([128, 8], u32, tag="i8")
                    nc.vector.max_index(out=i8[:], in_max=m8[:], in_values=h3[:])
                    nc.scalar.dma_start(cand[rs, :], i8[:])
    if split_waits:
        _split_multi_waits(nc)
    return nc


def _get_nc():
    if "nc" not in _cache:
        _cache["nc"] = _build_nc()
    return _cache["nc"]


LAST_RESULTS = None  # BassKernelResults of the most recent device run


def kernel(x, fragment_environments, tp_w, critic_w, critic_b, masks):
    global LAST_RESULTS
    import ml_dtypes

    from concourse.bass_utils import run_bass_kernel_spmd

    bf16 = ml_dtypes.bfloat16

    x = np.asarray(x, np.float32)
    y = np.asarray(fragment_environments, np.float32)
    tp_w = np.asarray(tp_w, np.float32)
    critic_w = np.asarray(critic_w, np.float32)
    critic_b = np.asarray(critic_b, np.float32)
    masks = np.asarray(masks)

    # ---- host-side prep (tiny vs the [N, F] device work) ----
    A = _build_A(tp_w)  # [208, 208] f64
    xp64 = x.astype(np.float64) @ A  # [N, 208] f64
    xpT32 = np.ascontiguousarray(xp64.T.astype(np.float32))  # [208, N]
    xpTb = xpT32.astype(bf16)
    yTb = np.ascontiguousarray(y.T).astype(bf16)  # [208, F]
    wp = np.ascontiguousarray(
        (y.astype(np.float64).T @ critic_w.astype(np.float64)).astype(np.float32)
    )  # [208, 1]
    g = _gumbel()
    all_valid = bool(masks.all())
    if all_valid:
        gmb = g.astype(bf16)
    else:
        gmb = np.where(masks, g, np.float32(MASKPEN)).astype(bf16)
    identity = np.eye(128, dtype=np.float32).astype(bf16)

    in_maps = []
    for c in range(NCORES):
        rows = slice(c * RPC, (c + 1) * RPC)
        in_maps.append(
            {
                "xb": np.ascontiguousarray(xpTb[:, rows]),
                "yb": yTb,
                "gb": np.ascontiguousarray(gmb[rows]),
                "xf": np.ascontiguousarray(xpT32[:, rows]),
                "wp": wp,
                "identb": identity,
            }
        )

    nc = _get_nc()
    res = run_bass_kernel_spmd(nc, in_maps, core_ids=list(range(NCORES)))
    LAST_RESULTS = res

    cand = np.concatenate([r["cand"] for r in res.results], axis=0)  # [N, 8] u32
    value = np.concatenate([r["val"] for r in res.results], axis=0)  # [N, 1] f32

    # ---- host rescore of the <=64 candidate columns per row (exact f64) ----
    q = cand.astype(np.int64)
    q[q >= F // 8] = 0  # guard (unmatched top-8 slots would be 0xFFFFFFFF)
    # oct q covers columns (8*(q//512) + k)*512 + q%512, k in 0..7
    js = (
        (8 * (q[:, :, None] // JT) + np.arange(8)[None, None, :]) * JT
        + q[:, :, None] % JT
    ).reshape(N, 64)
    rows = np.arange(N)[:, None]
    f_cand = np.einsum(
        "nd,ncd->nc", xp64, y.astype(np.float64)[js], optimize=True
    )  # [N, 32] f64
    g_cand = g[rows, js].astype(np.float64)
    if all_valid:
        score = f_cand + g_cand
    else:
        # reference does fp32 `where(mask, f, MIN) + g`; for masked columns
        # MIN + g rounds to exactly MIN in fp32, so model that faithfully
        m_cand = masks[rows, js]
        score = np.where(m_cand, f_cand + g_cand, np.float64(F32MIN))
    best = score.max(axis=1, keepdims=True)
    jpick = np.where(score == best, js, np.int64(1 << 40))
    actions = jpick.min(axis=1).astype(np.int32)
    if not all_valid:
        # fully-masked rows: reference argmax over all-equal logits returns 0
        actions[best[:, 0] == np.float64(F32MIN)] = 0

    value = value + critic_b.astype(np.float32)  # [N, 1]
    return actions, value


if __name__ == "__main__":
    # smoke test with random data of the right shapes
    rng = np.random.default_rng(0)
    ins = {
        "x": rng.standard_normal((N, D)).astype(np.float32),
        "fragment_environments": rng.standard_normal((F, D)).astype(np.float32),
        "tp_w": rng.standard_normal((4, MUL, MUL)).astype(np.float32),
        "critic_w": (rng.standard_normal((F, 1)) / np.sqrt(F)).astype(np.float32),
        "critic_b": np.zeros((1,), dtype=np.float32),
        "masks": np.ones((N, F), dtype=bool),
    }
    a, v = kernel(**ins)
    print(a[:8], v[:4, 0])


# revision 25
# speedup vs baseline: 42.2187x; 1.1780x over previous
"""Trainium2 Bass kernel for nn_Agent_88828513616344 (gnn_message_passing).

Math (see reference): with A the [208,208] block matrix built from tp_w
(CG-scaled, block-diagonal per irrep l, diagonal in m), the pairwise tensor
product scores factorize as

    features = (x @ A) @ fragment_environments.T          # [N, F]
    actions  = argmax(where(masks, features, MIN) + g)    # gumbel-max, fixed key
    value    = features @ critic_w + critic_b = (x @ A) @ (fragment_environments.T @ critic_w) + b

Sharding: N=4096 rows split across 8 NeuronCores (data parallel); the
fragment-environment table, tp weights and critic are replicated.

Per-core device kernel (all sizes per core: 512 rows):
  - PE computes t = g + x' @ y^T into PSUM in bf16 (g is injected through an
    identity matmul so no vector-engine add is needed; PSUM accumulates fp32).
  - ScalarE copies PSUM tiles into a full [128, 8192] SBUF row buffer.
  - VectorE folds the row twice with fused pairwise max (tensor_tensor_reduce):
    8192 -> 4096 -> 2048 "quad maxes", then top-8 + indices (Max/MaxIndex).
  - The critic head (tiny [208]-contraction) runs in true fp32 on PE.
Device returns per row the top-8 quad indices (each quad = 4 candidate
columns). The host rescores those <=32 candidate columns per row exactly in
float64 — the bf16 feature error (~3e-3) is astronomically unlikely to push
the true gumbel-max winner out of the device's top-8 quads (the gumbel top-9
spread is O(1)), and the final argmax over candidates is exact.
"""

import numpy as np

MUL = 13
LMAX = 3
FAN_IN = 4 * MUL * MUL
N, F, D = 4096, 8192, 208
NCORES = 8
RPC = N // NCORES  # rows per core = 512
ICH = RPC // 128  # i-chunks per core = 4
JT = 512
NJT = F // JT  # 16
KSPLIT = 104  # 208 = 104 + 104 contraction split
F32MIN = float(np.finfo(np.float32).min)
MASKPEN = -3.0e38  # large-negative, finite in bf16 (avoids 0*inf NaN in the PE)

_cache: dict = {}


def _gumbel() -> np.ndarray:
    """The reference's deterministic gumbel noise (fixed key, fixed shape).

    Computed with the same call on the same (default) jax backend as the
    reference, so the bits match exactly — PRNG output differs between
    backends (trn2 vs cpu), so do NOT pin a device here.
    """
    if "g" not in _cache:
        import jax

        g = jax.random.gumbel(jax.random.key(42), (N, F), dtype=np.float32)
        _cache["g"] = np.asarray(g)
    return _cache["g"]


def _build_A(tp_w: np.ndarray) -> np.ndarray:
    """[208, 208] float64 block matrix: A[off+u*m+t, off+v*m+t] = c_l W_l[u,v]."""
    A = np.zeros((D, D), np.float64)
    off = 0
    for l in range(LMAX + 1):
        m = 2 * l + 1
        c = 1.0 / np.sqrt((2 * l + 1) * FAN_IN)
        W = tp_w[l].astype(np.float64) * c
        for t in range(m):
            A[off + t : off + MUL * m : m, off + t : off + MUL * m : m] = W
        off += MUL * m
    return A


def _pin_dma_lanes():
    """Pin HWDGE DMA-completion sems to one lane per issuing ring.

    Tile round-robins DMA completions over 8 DMAHW semaphore lanes, so an
    instruction first-consuming two DMA'd tiles needs two sync waits — but
    walrus rejects Matmult instructions with more than one sync wait ("Too
    many sync wait commands" in setupSyncWait<S3_LW>). HWDGE executes DMAs
    FIFO per issuing engine (two physical rings: SP and ACT), so pinning each
    ring to a single monotone lane is safe and collapses all DMA waits from
    one ring into a single `lane >= tick` wait.
    """
    if _cache.get("lanes_patched"):
        return
    import concourse.mybir as mybir
    import concourse.tile_sem_assignment as tsa
    from concourse.tile_scheduler import DMAInst

    lanes = {mybir.EngineType.SP: 0, mybir.EngineType.Activation: 1}
    orig = tsa.TileClockTick._assign_tick

    def pinned(self, inst):
        if isinstance(inst, DMAInst):
            lane = lanes.get(inst.engine)
            if lane is not None:
                self.next_hw_dma_idx = lane
        return orig(self, inst)

    tsa.TileClockTick._assign_tick = pinned
    _cache["lanes_patched"] = True


def _split_multi_waits(nc):
    """Walrus codegen allows only ONE sync wait per ISA instruction in this
    build. Move surplus waits of any scheduled instruction onto engine-local
    NoOps inserted directly before it (same engine stream => same semantics).
    """
    import concourse.mybir as mybir

    fn = nc.m.functions[0]
    k = 0
    for blk in fn.blocks:
        out = []
        for inst in blk.instructions:
            si = inst.sync_info
            waits = list(si.on_wait) if si is not None and si.on_wait else []
            if len(waits) > 1:
                for w in waits[:-1]:
                    nop = mybir.InstNoOp(name=f"I-wsplit-{k}", ins=[], outs=[])
                    k += 1
                    nop.engine = inst.engine
                    nop.sync_info = mybir.SyncInfo(on_wait=[w], on_update=[])
                    out.append(nop)
                inst.sync_info = mybir.SyncInfo(
                    on_wait=[waits[-1]], on_update=list(si.on_update or [])
                )
            out.append(inst)
        blk.instructions = out


def _build_nc(reps: int = 1, split_waits: bool = True, stage: int = 4, paired: bool = True):
    """stage: 1 = DMA only, 2 = +matmuls, 3 = +PSUM->SBUF copies, 4 = full.
    paired: use [128,1024] two-bank PSUM tiles (one ACT copy per 6 matmuls)."""
    import concourse.bass as bass
    import concourse.mybir as mybir
    import concourse.tile as tile

    _pin_dma_lanes()

    f32 = mybir.dt.float32
    bf16 = mybir.dt.bfloat16
    u32 = mybir.dt.uint32
    MAX = mybir.AluOpType.max

    nc = bass.Bass("TRN2", target_bir_lowering=False, debug=False)
    xb = nc.dram_tensor("xb", [D, RPC], bf16, kind="ExternalInput").ap()
    yb = nc.dram_tensor("yb", [D, F], bf16, kind="ExternalInput").ap()
    gb = nc.dram_tensor("gb", [RPC, F], bf16, kind="ExternalInput").ap()
    xf = nc.dram_tensor("xf", [D, RPC], f32, kind="ExternalInput").ap()
    wp = nc.dram_tensor("wp", [D, 1], f32, kind="ExternalInput").ap()
    identb = nc.dram_tensor("identb", [128, 128], bf16, kind="ExternalInput").ap()
    cand = nc.dram_tensor("cand", [RPC, 8], u32, kind="ExternalOutput").ap()
    val = nc.dram_tensor("val", [RPC, 1], f32, kind="ExternalOutput").ap()

    YC = 4  # load y in column chunks so early matmuls start sooner

    with tile.TileContext(nc) as tc:
        with (
            tc.tile_pool(name="const", bufs=1) as cpool,
            tc.tile_pool(name="gpool", bufs=3) as gpool,
            tc.tile_pool(name="tpool", bufs=2) as tpool,
            tc.tile_pool(name="hpool", bufs=2) as hpool,
            tc.tile_pool(name="small", bufs=3) as spool,
            tc.tile_pool(name="psum", bufs=3, space="PSUM") as ppool,
            tc.tile_pool(name="psumv", bufs=1, space="PSUM") as pvpool,
            tc.tile_pool(name="psumscr", bufs=1, space="PSUM") as pspool,
        ):
            y0 = cpool.tile([KSPLIT, F], bf16, tag="y0")
            y1 = cpool.tile([KSPLIT, F], bf16, tag="y1")
            x0 = cpool.tile([KSPLIT, RPC], bf16, tag="x0")
            x1 = cpool.tile([KSPLIT, RPC], bf16, tag="x1")
            xf0 = cpool.tile([KSPLIT, RPC], f32, tag="xf0")
            xf1 = cpool.tile([KSPLIT, RPC], f32, tag="xf1")
            w0 = cpool.tile([KSPLIT, 1], f32, tag="w0")
            w1 = cpool.tile([KSPLIT, 1], f32, tag="w1")
            Ib = cpool.tile([128, 128], bf16, tag="Ib")

            FC = F // YC
            for _rep in range(reps):
                # constants + y on the ACT HWDGE ring; g stream owns the SP ring
                nc.scalar.dma_start(Ib[:], identb[:, :])
                nc.scalar.dma_start(x0[:], xb[0:KSPLIT, :])
                nc.scalar.dma_start(x1[:], xb[KSPLIT : 2 * KSPLIT, :])
                cs0 = slice(0, FC)
                nc.scalar.dma_start(y0[:, cs0], yb[0:KSPLIT, cs0])
                nc.scalar.dma_start(y1[:, cs0], yb[KSPLIT : 2 * KSPLIT, cs0])
                nc.scalar.dma_start(xf0[:], xf[0:KSPLIT, :])
                nc.scalar.dma_start(xf1[:], xf[KSPLIT : 2 * KSPLIT, :])
                nc.scalar.dma_start(w0[:], wp[0:KSPLIT, :])
                nc.scalar.dma_start(w1[:], wp[KSPLIT : 2 * KSPLIT, :])
                for jc in range(1, YC):
                    cs = slice(jc * FC, (jc + 1) * FC)
                    nc.scalar.dma_start(y0[:, cs], yb[0:KSPLIT, cs])
                    nc.scalar.dma_start(y1[:, cs], yb[KSPLIT : 2 * KSPLIT, cs])

                scr = pspool.tile([128, 1], f32, tag="scr")
                for ic in range(ICH):
                    rs = slice(ic * 128, (ic + 1) * 128)
                    gt = gpool.tile([128, F], bf16, tag="gt")
                    nc.sync.dma_start(gt[:], gb[rs, :])

                    # observer matmul: absorbs the g-stream's DMA-lane wait
                    # into a dedicated never-read PSUM bank, so no real
                    # matmul ever needs two sync waits (walrus limit).
                    if stage >= 2:
                        nc.tensor.matmul(
                            scr[:], gt[:, 0:128], gt[:, 0:1], start=True, stop=True
                        )

                    # critic head in true fp32 (exact): val = x'[rows] @ wp
                    if stage >= 2:
                        pv = pvpool.tile([128, 1], f32, tag="pv")
                        nc.tensor.matmul(
                            pv[:], xf0[:, rs], w0[:], start=True, stop=False
                        )
                        nc.tensor.matmul(
                            pv[:], xf1[:, rs], w1[:], start=False, stop=True
                        )
                        vt = spool.tile([128, 1], f32, tag="vt")
                        nc.scalar.copy(vt[:], pv[:])
                        nc.scalar.dma_start(val[rs, :], vt[:])

                    # t = g + x' @ y^T (bf16 inputs, fp32 PSUM accumulation).
                    # g-inject matmul goes FIRST (start=True) so the PSUM
                    # WAR wait lands on an instruction with no fresh-DMA wait.
                    # Process j-tiles in pairs sharing one 2-bank PSUM tile:
                    # one ACT copy + one PSUM-WAR sem round-trip per 6 matmuls
                    # instead of per 3 (the per-tile sem latency was the
                    # dominant serialization on HW).
                    t = tpool.tile([128, F], bf16, tag="t")
                    if not paired:
                        for j in range(NJT):
                            if stage < 2:
                                continue
                            js = slice(j * JT, (j + 1) * JT)
                            ptj = ppool.tile([128, 2 * JT], f32, tag="pt")
                            pb = ptj[:, 0:JT]
                            nc.tensor.matmul(
                                pb, Ib[:], gt[:, js], start=True, stop=False
                            )
                            nc.tensor.matmul(
                                pb, x0[:, rs], y0[:, js], start=False, stop=False
                            )
                            nc.tensor.matmul(
                                pb, x1[:, rs], y1[:, js], start=False, stop=True
                            )
                            if stage >= 3:
                                nc.scalar.copy(t[:, js], pb)
                    for a in range(NJT // 2 if paired else 0):
                        if stage < 2:
                            continue
                        pt = ppool.tile([128, 2 * JT], f32, tag="pt")
                        js0 = slice((2 * a) * JT, (2 * a + 1) * JT)
                        js1 = slice((2 * a + 1) * JT, (2 * a + 2) * JT)
                        b0 = pt[:, 0:JT]
                        b1 = pt[:, JT : 2 * JT]
                        nc.tensor.matmul(b0, Ib[:], gt[:, js0], start=True, stop=False)
                        nc.tensor.matmul(b1, Ib[:], gt[:, js1], start=True, stop=False)
                        nc.tensor.matmul(
                            b0, x0[:, rs], y0[:, js0], start=False, stop=False
                        )
                        nc.tensor.matmul(
                            b1, x0[:, rs], y0[:, js1], start=False, stop=False
                        )
                        nc.tensor.matmul(
                            b0, x1[:, rs], y1[:, js0], start=False, stop=True
                        )
                        nc.tensor.matmul(
                            b1, x1[:, rs], y1[:, js1], start=False, stop=True
                        )
                        if stage >= 3:
                            nc.scalar.copy(
                                t[:, (2 * a) * JT : (2 * a + 2) * JT], pt[:]
                            )

                    if stage < 4:
                        continue
                    # adjacent-pair max fold (bf16 tensor_tensor -> 2x DVE mode)
                    #   h[a*512+o]  = max(t[2a*512+o],  t[(2a+1)*512+o])
                    #   h2[b*512+o] = max(h[2b*512+o],  h[(2b+1)*512+o])
                    #   h3[c*512+o] = max(h2[2c*512+o], h2[(2c+1)*512+o])
                    # -> oct r covers columns (8*(r//512)+k)*512 + r%512, k<8
                    h = hpool.tile([128, F // 2], bf16, tag="h")
                    for a in range(8):
                        nc.vector.tensor_tensor(
                            h[:, a * JT : (a + 1) * JT],
                            t[:, (2 * a) * JT : (2 * a + 1) * JT],
                            t[:, (2 * a + 1) * JT : (2 * a + 2) * JT],
                            MAX,
                        )
                    h2 = hpool.tile([128, F // 4], bf16, tag="h2")
                    for b in range(4):
                        nc.vector.tensor_tensor(
                            h2[:, b * JT : (b + 1) * JT],
                            h[:, (2 * b) * JT : (2 * b + 1) * JT],
                            h[:, (2 * b + 1) * JT : (2 * b + 2) * JT],
                            MAX,
                        )
                    h3 = hpool.tile([128, F // 8], bf16, tag="h3")
                    for c in range(2):
                        nc.vector.tensor_tensor(
                            h3[:, c * JT : (c + 1) * JT],
                            h2[:, (2 * c) * JT : (2 * c + 1) * JT],
                            h2[:, (2 * c + 1) * JT : (2 * c + 2) * JT],
                            MAX,
                        )
                    m8 = spool.tile([128, 8], bf16, tag="m8")
                    nc.vector.max(out=m8[:], in_=h3[:])
                    i8 = spool.tile([128, 8], u32, tag="i8")
                    nc.vector.max_index(out=i8[:], in_max=m8[:], in_values=h3[:])
                    nc.scalar.dma_start(cand[rs, :], i8[:])
    if split_waits:
        _split_multi_waits(nc)
    return nc


def _get_nc():
    if "nc" not in _cache:
        _cache["nc"] = _build_nc()
    return _cache["nc"]


LAST_RESULTS = None  # BassKernelResults of the most recent device run


def kernel(x, fragment_environments, tp_w, critic_w, critic_b, masks):
    global LAST_RESULTS
    import ml_dtypes

    from concourse.bass_utils import run_bass_kernel_spmd

    bf16 = ml_dtypes.bfloat16

    x = np.asarray(x, np.float32)
    y = np.asarray(fragment_environments, np.float32)
    tp_w = np.asarray(tp_w, np.float32)
    critic_w = np.asarray(critic_w, np.float32)
    critic_b = np.asarray(critic_b, np.float32)
    masks = np.asarray(masks)

    # ---- host-side prep (tiny vs the [N, F] device work) ----
    A = _build_A(tp_w)  # [208, 208] f64
    xp64 = x.astype(np.float64) @ A  # [N, 208] f64
    xpT32 = np.ascontiguousarray(xp64.T.astype(np.float32))  # [208, N]
    xpTb = xpT32.astype(bf16)
    yTb = np.ascontiguousarray(y.T).astype(bf16)  # [208, F]
    wp = np.ascontiguousarray(
        (y.astype(np.float64).T @ critic_w.astype(np.float64)).astype(np.float32)
    )  # [208, 1]
    g = _gumbel()
    all_valid = bool(masks.all())
    if all_valid:
        gmb = g.astype(bf16)
    else:
        gmb = np.where(masks, g, np.float32(MASKPEN)).astype(bf16)
    identity = np.eye(128, dtype=np.float32).astype(bf16)

    in_maps = []
    for c in range(NCORES):
        rows = slice(c * RPC, (c + 1) * RPC)
        in_maps.append(
            {
                "xb": np.ascontiguousarray(xpTb[:, rows]),
                "yb": yTb,
                "gb": np.ascontiguousarray(gmb[rows]),
                "xf": np.ascontiguousarray(xpT32[:, rows]),
                "wp": wp,
                "identb": identity,
            }
        )

    nc = _get_nc()
    res = run_bass_kernel_spmd(nc, in_maps, core_ids=list(range(NCORES)))
    LAST_RESULTS = res

    cand = np.concatenate([r["cand"] for r in res.results], axis=0)  # [N, 8] u32
    value = np.concatenate([r["val"] for r in res.results], axis=0)  # [N, 1] f32

    # ---- host rescore of the <=64 candidate columns per row (exact f64) ----
    q = cand.astype(np.int64)
    q[q >= F // 8] = 0  # guard (unmatched top-8 slots would be 0xFFFFFFFF)
    # oct q covers columns (8*(q//512) + k)*512 + q%512, k in 0..7
    js = (
        (8 * (q[:, :, None] // JT) + np.arange(8)[None, None, :]) * JT
        + q[:, :, None] % JT
    ).reshape(N, 64)
    rows = np.arange(N)[:, None]
    f_cand = np.einsum(
        "nd,ncd->nc", xp64, y.astype(np.float64)[js], optimize=True
    )  # [N, 32] f64
    g_cand = g[rows, js].astype(np.float64)
    if all_valid:
        score = f_cand + g_cand
    else:
        # reference does fp32 `where(mask, f, MIN) + g`; for masked columns
        # MIN + g rounds to exactly MIN in fp32, so model that faithfully
        m_cand = masks[rows, js]
        score = np.where(m_cand, f_cand + g_cand, np.float64(F32MIN))
    best = score.max(axis=1, keepdims=True)
    jpick = np.where(score == best, js, np.int64(1 << 40))
    actions = jpick.min(axis=1).astype(np.int32)
    if not all_valid:
        # fully-masked rows: reference argmax over all-equal logits returns 0
        actions[best[:, 0] == np.float64(F32MIN)] = 0

    value = value + critic_b.astype(np.float32)  # [N, 1]
    return actions, value


if __name__ == "__main__":
    # smoke test with random data of the right shapes
    rng = np.random.default_rng(0)
    ins = {
        "x": rng.standard_normal((N, D)).astype(np.float32),
        "fragment_environments": rng.standard_normal((F, D)).astype(np.float32),
        "tp_w": rng.standard_normal((4, MUL, MUL)).astype(np.float32),
        "critic_w": (rng.standard_normal((F, 1)) / np.sqrt(F)).astype(np.float32),
        "critic_b": np.zeros((1,), dtype=np.float32),
        "masks": np.ones((N, F), dtype=bool),
    }
    a, v = kernel(**ins)
    print(a[:8], v[:4, 0])


# revision 30
# speedup vs baseline: 44.3170x; 1.0497x over previous
"""Trainium2 Bass kernel for nn_Agent_88828513616344 (gnn_message_passing).

Math (see reference): with A the [208,208] block matrix built from tp_w
(CG-scaled, block-diagonal per irrep l, diagonal in m), the pairwise tensor
product scores factorize as

    features = (x @ A) @ fragment_environments.T          # [N, F]
    actions  = argmax(where(masks, features, MIN) + g)    # gumbel-max, fixed key
    value    = features @ critic_w + critic_b = (x @ A) @ (fragment_environments.T @ critic_w) + b

Sharding: N=4096 rows split across 8 NeuronCores (data parallel); the
fragment-environment table, tp weights and critic are replicated.

Per-core device kernel (all sizes per core: 512 rows):
  - PE computes t = g + x' @ y^T into PSUM in bf16 (g is injected through an
    identity matmul so no vector-engine add is needed; PSUM accumulates fp32).
  - ScalarE copies PSUM tiles into a full [128, 8192] SBUF row buffer.
  - VectorE folds the row twice with fused pairwise max (tensor_tensor_reduce):
    8192 -> 4096 -> 2048 "quad maxes", then top-8 + indices (Max/MaxIndex).
  - The critic head (tiny [208]-contraction) runs in true fp32 on PE.
Device returns per row the top-8 quad indices (each quad = 4 candidate
columns). The host rescores those <=32 candidate columns per row exactly in
float64 — the bf16 feature error (~3e-3) is astronomically unlikely to push
the true gumbel-max winner out of the device's top-8 quads (the gumbel top-9
spread is O(1)), and the final argmax over candidates is exact.
"""

import numpy as np

MUL = 13
LMAX = 3
FAN_IN = 4 * MUL * MUL
N, F, D = 4096, 8192, 208
NCORES = 8
RPC = N // NCORES  # rows per core = 512
ICH = RPC // 128  # i-chunks per core = 4
JT = 512
NJT = F // JT  # 16
KSPLIT = 104  # 208 = 104 + 104 contraction split
F32MIN = float(np.finfo(np.float32).min)
MASKPEN = -3.0e38  # large-negative, finite in bf16 (avoids 0*inf NaN in the PE)

_cache: dict = {}


def _gumbel() -> np.ndarray:
    """The reference's deterministic gumbel noise (fixed key, fixed shape).

    Computed with the same call on the same (default) jax backend as the
    reference, so the bits match exactly — PRNG output differs between
    backends (trn2 vs cpu), so do NOT pin a device here.
    """
    if "g" not in _cache:
        import jax

        g = jax.random.gumbel(jax.random.key(42), (N, F), dtype=np.float32)
        _cache["g"] = np.asarray(g)
    return _cache["g"]


def _build_A(tp_w: np.ndarray) -> np.ndarray:
    """[208, 208] float64 block matrix: A[off+u*m+t, off+v*m+t] = c_l W_l[u,v]."""
    A = np.zeros((D, D), np.float64)
    off = 0
    for l in range(LMAX + 1):
        m = 2 * l + 1
        c = 1.0 / np.sqrt((2 * l + 1) * FAN_IN)
        W = tp_w[l].astype(np.float64) * c
        for t in range(m):
            A[off + t : off + MUL * m : m, off + t : off + MUL * m : m] = W
        off += MUL * m
    return A


def _pin_dma_lanes():
    """Pin HWDGE DMA-completion sems to one lane per issuing ring.

    Tile round-robins DMA completions over 8 DMAHW semaphore lanes, so an
    instruction first-consuming two DMA'd tiles needs two sync waits — but
    walrus rejects Matmult instructions with more than one sync wait ("Too
    many sync wait commands" in setupSyncWait<S3_LW>). HWDGE executes DMAs
    FIFO per issuing engine (two physical rings: SP and ACT), so pinning each
    ring to a single monotone lane is safe and collapses all DMA waits from
    one ring into a single `lane >= tick` wait.
    """
    if _cache.get("lanes_patched"):
        return
    import concourse.mybir as mybir
    import concourse.tile_sem_assignment as tsa
    from concourse.tile_scheduler import DMAInst

    lanes = {mybir.EngineType.SP: 0, mybir.EngineType.Activation: 1}
    orig = tsa.TileClockTick._assign_tick

    def pinned(self, inst):
        if isinstance(inst, DMAInst):
            lane = lanes.get(inst.engine)
            if lane is not None:
                self.next_hw_dma_idx = lane
        return orig(self, inst)

    tsa.TileClockTick._assign_tick = pinned
    _cache["lanes_patched"] = True


def _split_multi_waits(nc):
    """Walrus codegen allows only ONE sync wait per ISA instruction in this
    build. Move surplus waits of any scheduled instruction onto engine-local
    NoOps inserted directly before it (same engine stream => same semantics).
    """
    import concourse.mybir as mybir

    fn = nc.m.functions[0]
    k = 0
    for blk in fn.blocks:
        out = []
        for inst in blk.instructions:
            si = inst.sync_info
            waits = list(si.on_wait) if si is not None and si.on_wait else []
            if len(waits) > 1:
                for w in waits[:-1]:
                    nop = mybir.InstNoOp(name=f"I-wsplit-{k}", ins=[], outs=[])
                    k += 1
                    nop.engine = inst.engine
                    nop.sync_info = mybir.SyncInfo(on_wait=[w], on_update=[])
                    out.append(nop)
                inst.sync_info = mybir.SyncInfo(
                    on_wait=[waits[-1]], on_update=list(si.on_update or [])
                )
            out.append(inst)
        blk.instructions = out


def _build_nc(
    reps: int = 1,
    split_waits: bool = True,
    stage: int = 4,
    paired: bool = True,
    dve_g: int = 0,
    deep_bufs: bool = False,
):
    """stage: 1 = DMA only, 2 = +matmuls, 3 = +PSUM->SBUF copies, 4 = full.
    paired: use [128,1024] two-bank PSUM tiles (one ACT copy per 6 matmuls).
    dve_g: for this many tile-pairs per chunk, inject g + evacuate PSUM via a
    single DVE tensor_tensor add instead of PE identity-matmuls + ACT copy.
    deep_bufs: t bufs 3 / g bufs 4 for extra cross-chunk pipeline slack."""
    import concourse.bass as bass
    import concourse.mybir as mybir
    import concourse.tile as tile

    _pin_dma_lanes()

    f32 = mybir.dt.float32
    bf16 = mybir.dt.bfloat16
    u32 = mybir.dt.uint32
    MAX = mybir.AluOpType.max
    ADD = mybir.AluOpType.add

    nc = bass.Bass("TRN2", target_bir_lowering=False, debug=False)
    xb = nc.dram_tensor("xb", [D, RPC], bf16, kind="ExternalInput").ap()
    yb = nc.dram_tensor("yb", [D, F], bf16, kind="ExternalInput").ap()
    gb = nc.dram_tensor("gb", [RPC, F], bf16, kind="ExternalInput").ap()
    xf = nc.dram_tensor("xf", [D, RPC], f32, kind="ExternalInput").ap()
    wp = nc.dram_tensor("wp", [D, 1], f32, kind="ExternalInput").ap()
    identb = nc.dram_tensor("identb", [128, 128], bf16, kind="ExternalInput").ap()
    cand = nc.dram_tensor("cand", [RPC, 8], u32, kind="ExternalOutput").ap()
    val = nc.dram_tensor("val", [RPC, 1], f32, kind="ExternalOutput").ap()

    YC = 4  # load y in column chunks so early matmuls start sooner

    with tile.TileContext(nc) as tc:
        with (
            tc.tile_pool(name="const", bufs=1) as cpool,
            tc.tile_pool(name="gpool", bufs=4 if deep_bufs else 3) as gpool,
            tc.tile_pool(name="tpool", bufs=3 if deep_bufs else 2) as tpool,
            tc.tile_pool(name="hpool", bufs=2) as hpool,
            tc.tile_pool(name="small", bufs=3) as spool,
            tc.tile_pool(name="psum", bufs=3, space="PSUM") as ppool,
            tc.tile_pool(name="psumv", bufs=1, space="PSUM") as pvpool,
            tc.tile_pool(name="psumscr", bufs=1, space="PSUM") as pspool,
        ):
            y0 = cpool.tile([KSPLIT, F], bf16, tag="y0")
            y1 = cpool.tile([KSPLIT, F], bf16, tag="y1")
            x0 = cpool.tile([KSPLIT, RPC], bf16, tag="x0")
            x1 = cpool.tile([KSPLIT, RPC], bf16, tag="x1")
            xf0 = cpool.tile([KSPLIT, RPC], f32, tag="xf0")
            xf1 = cpool.tile([KSPLIT, RPC], f32, tag="xf1")
            w0 = cpool.tile([KSPLIT, 1], f32, tag="w0")
            w1 = cpool.tile([KSPLIT, 1], f32, tag="w1")
            Ib = cpool.tile([128, 128], bf16, tag="Ib")

            FC = F // YC
            for _rep in range(reps):
                # constants + y on the ACT HWDGE ring; g stream owns the SP ring
                nc.scalar.dma_start(Ib[:], identb[:, :])
                nc.scalar.dma_start(x0[:], xb[0:KSPLIT, :])
                nc.scalar.dma_start(x1[:], xb[KSPLIT : 2 * KSPLIT, :])
                cs0 = slice(0, FC)
                nc.scalar.dma_start(y0[:, cs0], yb[0:KSPLIT, cs0])
                nc.scalar.dma_start(y1[:, cs0], yb[KSPLIT : 2 * KSPLIT, cs0])
                nc.scalar.dma_start(xf0[:], xf[0:KSPLIT, :])
                nc.scalar.dma_start(xf1[:], xf[KSPLIT : 2 * KSPLIT, :])
                nc.scalar.dma_start(w0[:], wp[0:KSPLIT, :])
                nc.scalar.dma_start(w1[:], wp[KSPLIT : 2 * KSPLIT, :])
                for jc in range(1, YC):
                    cs = slice(jc * FC, (jc + 1) * FC)
                    nc.scalar.dma_start(y0[:, cs], yb[0:KSPLIT, cs])
                    nc.scalar.dma_start(y1[:, cs], yb[KSPLIT : 2 * KSPLIT, cs])

                scr = pspool.tile([128, 1], f32, tag="scr")
                for ic in range(ICH):
                    rs = slice(ic * 128, (ic + 1) * 128)
                    gt = gpool.tile([128, F], bf16, tag="gt")
                    nc.sync.dma_start(gt[:], gb[rs, :])

                    # observer matmul: absorbs the g-stream's DMA-lane wait
                    # into a dedicated never-read PSUM bank, so no real
                    # matmul ever needs two sync waits (walrus limit).
                    if stage >= 2:
                        nc.tensor.matmul(
                            scr[:], gt[:, 0:128], gt[:, 0:1], start=True, stop=True
                        )

                    # critic head in true fp32 (exact): val = x'[rows] @ wp
                    if stage >= 2:
                        pv = pvpool.tile([128, 1], f32, tag="pv")
                        nc.tensor.matmul(
                            pv[:], xf0[:, rs], w0[:], start=True, stop=False
                        )
                        nc.tensor.matmul(
                            pv[:], xf1[:, rs], w1[:], start=False, stop=True
                        )
                        vt = spool.tile([128, 1], f32, tag="vt")
                        nc.scalar.copy(vt[:], pv[:])
                        nc.scalar.dma_start(val[rs, :], vt[:])

                    # t = g + x' @ y^T (bf16 inputs, fp32 PSUM accumulation).
                    # g-inject matmul goes FIRST (start=True) so the PSUM
                    # WAR wait lands on an instruction with no fresh-DMA wait.
                    # Process j-tiles in pairs sharing one 2-bank PSUM tile:
                    # one ACT copy + one PSUM-WAR sem round-trip per 6 matmuls
                    # instead of per 3 (the per-tile sem latency was the
                    # dominant serialization on HW).
                    t = tpool.tile([128, F], bf16, tag="t")
                    if not paired:
                        for j in range(NJT):
                            if stage < 2:
                                continue
                            js = slice(j * JT, (j + 1) * JT)
                            ptj = ppool.tile([128, 2 * JT], f32, tag="pt")
                            pb = ptj[:, 0:JT]
                            nc.tensor.matmul(
                                pb, Ib[:], gt[:, js], start=True, stop=False
                            )
                            nc.tensor.matmul(
                                pb, x0[:, rs], y0[:, js], start=False, stop=False
                            )
                            nc.tensor.matmul(
                                pb, x1[:, rs], y1[:, js], start=False, stop=True
                            )
                            if stage >= 3:
                                nc.scalar.copy(t[:, js], pb)
                    for a in range(NJT // 2 if paired else 0):
                        if stage < 2:
                            continue
                        pt = ppool.tile([128, 2 * JT], f32, tag="pt")
                        js0 = slice((2 * a) * JT, (2 * a + 1) * JT)
                        js1 = slice((2 * a + 1) * JT, (2 * a + 2) * JT)
                        jsp = slice((2 * a) * JT, (2 * a + 2) * JT)
                        b0 = pt[:, 0:JT]
                        b1 = pt[:, JT : 2 * JT]
                        offload = a < dve_g
                        if not offload:
                            nc.tensor.matmul(
                                b0, Ib[:], gt[:, js0], start=True, stop=False
                            )
                            nc.tensor.matmul(
                                b1, Ib[:], gt[:, js1], start=True, stop=False
                            )
                        nc.tensor.matmul(
                            b0, x0[:, rs], y0[:, js0], start=offload, stop=False
                        )
                        nc.tensor.matmul(
                            b1, x0[:, rs], y0[:, js1], start=offload, stop=False
                        )
                        nc.tensor.matmul(
                            b0, x1[:, rs], y1[:, js0], start=False, stop=True
                        )
                        nc.tensor.matmul(
                            b1, x1[:, rs], y1[:, js1], start=False, stop=True
                        )
                        if stage >= 3:
                            if offload:
                                # g-inject + PSUM evacuation fused on DVE
                                nc.vector.tensor_tensor(
                                    t[:, jsp], pt[:], gt[:, jsp], ADD
                                )
                            else:
                                nc.scalar.copy(t[:, jsp], pt[:])

                    if stage < 4:
                        continue
                    # adjacent-pair max fold (bf16 tensor_tensor -> 2x DVE mode)
                    #   h[a*512+o]  = max(t[2a*512+o],  t[(2a+1)*512+o])
                    #   h2[b*512+o] = max(h[2b*512+o],  h[(2b+1)*512+o])
                    #   h3[c*512+o] = max(h2[2c*512+o], h2[(2c+1)*512+o])
                    # -> oct r covers columns (8*(r//512)+k)*512 + r%512, k<8
                    h = hpool.tile([128, F // 2], bf16, tag="h")
                    for a in range(8):
                        nc.vector.tensor_tensor(
                            h[:, a * JT : (a + 1) * JT],
                            t[:, (2 * a) * JT : (2 * a + 1) * JT],
                            t[:, (2 * a + 1) * JT : (2 * a + 2) * JT],
                            MAX,
                        )
                    h2 = hpool.tile([128, F // 4], bf16, tag="h2")
                    for b in range(4):
                        nc.vector.tensor_tensor(
                            h2[:, b * JT : (b + 1) * JT],
                            h[:, (2 * b) * JT : (2 * b + 1) * JT],
                            h[:, (2 * b + 1) * JT : (2 * b + 2) * JT],
                            MAX,
                        )
                    h3 = hpool.tile([128, F // 8], bf16, tag="h3")
                    for c in range(2):
                        nc.vector.tensor_tensor(
                            h3[:, c * JT : (c + 1) * JT],
                            h2[:, (2 * c) * JT : (2 * c + 1) * JT],
                            h2[:, (2 * c + 1) * JT : (2 * c + 2) * JT],
                            MAX,
                        )
                    m8 = spool.tile([128, 8], bf16, tag="m8")
                    nc.vector.max(out=m8[:], in_=h3[:])
                    i8 = spool.tile([128, 8], u32, tag="i8")
                    nc.vector.max_index(out=i8[:], in_max=m8[:], in_values=h3[:])
                    nc.scalar.dma_start(cand[rs, :], i8[:])
    if split_waits:
        _split_multi_waits(nc)
    return nc


BEST_KW = dict(dve_g=2, deep_bufs=True)  # fastest measured config (A/B, 2026-08-03)


def _get_nc():
    if "nc" not in _cache:
        _cache["nc"] = _build_nc(**BEST_KW)
    return _cache["nc"]


LAST_RESULTS = None  # BassKernelResults of the most recent device run


def kernel(x, fragment_environments, tp_w, critic_w, critic_b, masks):
    global LAST_RESULTS
    import ml_dtypes

    from concourse.bass_utils import run_bass_kernel_spmd

    bf16 = ml_dtypes.bfloat16

    x = np.asarray(x, np.float32)
    y = np.asarray(fragment_environments, np.float32)
    tp_w = np.asarray(tp_w, np.float32)
    critic_w = np.asarray(critic_w, np.float32)
    critic_b = np.asarray(critic_b, np.float32)
    masks = np.asarray(masks)

    # ---- host-side prep (tiny vs the [N, F] device work) ----
    A = _build_A(tp_w)  # [208, 208] f64
    xp64 = x.astype(np.float64) @ A  # [N, 208] f64
    xpT32 = np.ascontiguousarray(xp64.T.astype(np.float32))  # [208, N]
    xpTb = xpT32.astype(bf16)
    yTb = np.ascontiguousarray(y.T).astype(bf16)  # [208, F]
    wp = np.ascontiguousarray(
        (y.astype(np.float64).T @ critic_w.astype(np.float64)).astype(np.float32)
    )  # [208, 1]
    g = _gumbel()
    all_valid = bool(masks.all())
    if all_valid:
        gmb = g.astype(bf16)
    else:
        gmb = np.where(masks, g, np.float32(MASKPEN)).astype(bf16)
    identity = np.eye(128, dtype=np.float32).astype(bf16)

    in_maps = []
    for c in range(NCORES):
        rows = slice(c * RPC, (c + 1) * RPC)
        in_maps.append(
            {
                "xb": np.ascontiguousarray(xpTb[:, rows]),
                "yb": yTb,
                "gb": np.ascontiguousarray(gmb[rows]),
                "xf": np.ascontiguousarray(xpT32[:, rows]),
                "wp": wp,
                "identb": identity,
            }
        )

    nc = _get_nc()
    res = run_bass_kernel_spmd(nc, in_maps, core_ids=list(range(NCORES)))
    LAST_RESULTS = res

    cand = np.concatenate([r["cand"] for r in res.results], axis=0)  # [N, 8] u32
    value = np.concatenate([r["val"] for r in res.results], axis=0)  # [N, 1] f32

    # ---- host rescore of the <=64 candidate columns per row (exact f64) ----
    q = cand.astype(np.int64)
    q[q >= F // 8] = 0  # guard (unmatched top-8 slots would be 0xFFFFFFFF)
    # oct q covers columns (8*(q//512) + k)*512 + q%512, k in 0..7
    js = (
        (8 * (q[:, :, None] // JT) + np.arange(8)[None, None, :]) * JT
        + q[:, :, None] % JT
    ).reshape(N, 64)
    rows = np.arange(N)[:, None]
    f_cand = np.einsum(
        "nd,ncd->nc", xp64, y.astype(np.float64)[js], optimize=True
    )  # [N, 32] f64
    g_cand = g[rows, js].astype(np.float64)
    if all_valid:
        score = f_cand + g_cand
    else:
        # reference does fp32 `where(mask, f, MIN) + g`; for masked columns
        # MIN + g rounds to exactly MIN in fp32, so model that faithfully
        m_cand = masks[rows, js]
        score = np.where(m_cand, f_cand + g_cand, np.float64(F32MIN))
    best = score.max(axis=1, keepdims=True)
    jpick = np.where(score == best, js, np.int64(1 << 40))
    actions = jpick.min(axis=1).astype(np.int32)
    if not all_valid:
        # fully-masked rows: reference argmax over all-equal logits returns 0
        actions[best[:, 0] == np.float64(F32MIN)] = 0

    value = value + critic_b.astype(np.float32)  # [N, 1]
    return actions, value


if __name__ == "__main__":
    # smoke test with random data of the right shapes
    rng = np.random.default_rng(0)
    ins = {
        "x": rng.standard_normal((N, D)).astype(np.float32),
        "fragment_environments": rng.standard_normal((F, D)).astype(np.float32),
        "tp_w": rng.standard_normal((4, MUL, MUL)).astype(np.float32),
        "critic_w": (rng.standard_normal((F, 1)) / np.sqrt(F)).astype(np.float32),
        "critic_b": np.zeros((1,), dtype=np.float32),
        "masks": np.ones((N, F), dtype=bool),
    }
    a, v = kernel(**ins)
    print(a[:8], v[:4, 0])
